# revision 9
# baseline (speedup 1.0000x reference)
"""Trainium2 Bass kernel for nn_CodePredBlock (dense transformer block).

Sharding (8 cores): core c -> batch b = c//4, group g = c%4.
  - Attention: tensor-parallel over heads within each batch's 4-core group
    (4 q heads + 2 kv heads per core, GQA groups intact).
  - O-proj produces a per-core partial [T, H]; ReduceScatter over the
    4-core group yields each core's 512-token slice, fully summed.
  - FFN: token-parallel (512 tokens/core, full DFF), residuals fused.
All matmuls bf16 with fp32 accumulation. Norm weights and the attention
scale are folded into weights / RoPE tables on the host.

Device layouts: activations are feature-on-partition ("T" suffix means
transposed, [feature, token]); scores are computed transposed [tk, tq]
so softmax probs feed the V-matmul directly; softmax denominators and
rmsnorm partition reductions use ones-matmuls (which also broadcast).
"""

import os
import sys
from dataclasses import dataclass

import numpy as np

for _p in ("/opt/trn_rl_repo", "/root/.axon_site/_ro/trn_rl_repo"):
    if os.path.isdir(_p) and _p not in sys.path:
        sys.path.insert(0, _p)

import ml_dtypes  # noqa: E402

import concourse.bass as bass  # noqa: E402  (re-exported for callers)
import concourse.mybir as mybir  # noqa: E402
import concourse.tile as tile  # noqa: E402
from concourse import bacc  # noqa: E402
from concourse.masks import make_identity  # noqa: E402

F32 = mybir.dt.float32
BF16 = mybir.dt.bfloat16
AF = mybir.ActivationFunctionType
ALU = mybir.AluOpType
BF16NP = ml_dtypes.bfloat16

EPS = 1e-6
NEG = -1e9


@dataclass(frozen=True)
class Cfg:
    T: int = 2048          # sequence length
    H: int = 2048          # hidden
    DFF: int = 8192        # ffn intermediate (full)
    QH: int = 4            # q heads per core
    KVH: int = 2           # kv heads per core
    HD: int = 128          # head dim (must be 128)
    GROUP: int = 4         # cores per batch (tensor-parallel group)
    NCORES: int = 8
    TQ: int = 512          # q-token chunk for attention
    mask_mode: str = "causal"   # "causal" | "none" | "generic"

    @property
    def HSUB(self):
        return self.H // 128

    @property
    def TT(self):
        return self.T // 128

    @property
    def NQC(self):
        return self.T // self.TQ

    @property
    def TPC(self):
        return self.TQ // 128

    @property
    def TFFN(self):
        return self.T // self.GROUP

    @property
    def TF(self):
        return self.TFFN // 128

    @property
    def QCOLS(self):
        return self.QH * self.HD

    @property
    def KVCOLS(self):
        return self.KVH * self.HD


def build(cfg: Cfg, no_cc: bool = False):
    """Build + compile the SPMD Bass program (same program on all cores)."""
    c = cfg
    nc = bacc.Bacc("TRN2", target_bir_lowering=False, debug=False,
                   num_devices=c.NCORES)

    # ---- I/O ----
    xT = nc.dram_tensor("xT", [c.H, c.T], F32, kind="ExternalInput")
    x_res = nc.dram_tensor("x_res", [c.TFFN, c.H], F32, kind="ExternalInput")
    wqT = nc.dram_tensor("wqT", [c.H, c.QCOLS], BF16, kind="ExternalInput")
    wkT = nc.dram_tensor("wkT", [c.H, c.KVCOLS], BF16, kind="ExternalInput")
    wvT = nc.dram_tensor("wvT", [c.H, c.KVCOLS], BF16, kind="ExternalInput")
    woT = nc.dram_tensor("woT", [c.QCOLS, c.H], BF16, kind="ExternalInput")
    wgT = nc.dram_tensor("wgT", [c.H, c.DFF], BF16, kind="ExternalInput")
    wuT = nc.dram_tensor("wuT", [c.H, c.DFF], BF16, kind="ExternalInput")
    wdT = nc.dram_tensor("wdT", [c.DFF, c.H], BF16, kind="ExternalInput")
    cq = nc.dram_tensor("cq", [128, c.T], BF16, kind="ExternalInput")
    sq = nc.dram_tensor("sq", [128, c.T], BF16, kind="ExternalInput")
    ck = nc.dram_tensor("ck", [128, c.T], BF16, kind="ExternalInput")
    sk = nc.dram_tensor("sk", [128, c.T], BF16, kind="ExternalInput")
    if c.mask_mode == "causal":
        dmask = nc.dram_tensor("dmask", [c.TPC * 128, c.TQ], F32,
                               kind="ExternalInput")
    elif c.mask_mode == "generic":
        maskT = nc.dram_tensor("maskT", [c.T, c.T], F32, kind="ExternalInput")
    out = nc.dram_tensor("out", [c.TFFN, c.H], F32, kind="ExternalOutput")

    groups = [list(range(g * c.GROUP, (g + 1) * c.GROUP))
              for g in range(c.NCORES // c.GROUP)]

    def n_tk(qc):  # number of k tiles for q-chunk qc
        if c.mask_mode == "causal":
            return c.TPC * (qc + 1)
        return c.TT

    with tile.TileContext(nc) as tc:
        with (
            tc.tile_pool(name="dram", bufs=1, space="DRAM") as dram,
            tc.tile_pool(name="consts", bufs=1) as consts,
        ):
            o_part = dram.tile([c.T, c.H], F32)
            o_red = dram.tile([c.TFFN, c.H], F32)

            ones_f = consts.tile([128, 128], F32)
            nc.vector.memset(ones_f, 1.0)
            ones_b = consts.tile([128, 128], BF16)
            nc.vector.memset(ones_b, 1.0)
            ident = consts.tile([128, 128], BF16)
            make_identity(nc, ident)
            eps_sb = consts.tile([128, 1], F32)
            nc.vector.memset(eps_sb, EPS)

            # ---------------- attention phases ----------------
            with (
                tc.tile_pool(name="qkv", bufs=1) as qkv,      # q/k/v/ctx
            ):
                qbf = qkv.tile([128, c.QH, c.T], BF16)        # roped q^T
                kbf = qkv.tile([128, c.KVH, c.T], BF16)       # roped k^T
                vbf = qkv.tile([128, c.TT, c.KVCOLS], BF16)   # v token-major
                ctxb = qkv.tile([128, c.QH, c.T], BF16)       # ctx^T

                with tc.tile_pool(name="xn", bufs=1) as xn:
                    xnt = xn.tile([128, c.HSUB, c.T], BF16)   # x_norm^T

                    # -- phase 0: attn rmsnorm (feature-on-partition) --
                    with (
                        tc.tile_pool(name="p0", bufs=2) as p0,
                        tc.tile_pool(name="p0ps", bufs=1, space="PSUM") as p0ps,
                    ):
                        nss = c.T // c.TQ
                        ssq_ps = [p0ps.tile([128, c.TQ], F32, tag="ssq",
                                            bufs=nss, name=f"ssq_{i}")
                                  for i in range(nss)]
                        for s in range(c.HSUB):
                            xt_t = p0.tile([128, c.T], F32, tag="xt")
                            nc.sync.dma_start(
                                xt_t, xT[s * 128:(s + 1) * 128, :])
                            xsq = p0.tile([128, c.T], F32, tag="xsq")
                            nc.vector.tensor_mul(xsq, xt_t, xt_t)
                            for q in range(nss):
                                nc.tensor.matmul(
                                    ssq_ps[q], ones_f,
                                    xsq[:, q * c.TQ:(q + 1) * c.TQ],
                                    start=(s == 0), stop=(s == c.HSUB - 1))
                        inv_b = p0.tile([128, c.T], F32, tag="invb", bufs=1)
                        for q in range(nss):
                            sl = slice(q * c.TQ, (q + 1) * c.TQ)
                            rms = p0.tile([128, c.TQ], F32, tag="rms")
                            nc.scalar.activation(rms, ssq_ps[q], AF.Sqrt,
                                                 bias=eps_sb, scale=1.0 / c.H)
                            nc.vector.reciprocal(inv_b[:, sl], rms)
                        for s in range(c.HSUB):
                            xt_t = p0.tile([128, c.T], F32, tag="xt")
                            nc.sync.dma_start(
                                xt_t, xT[s * 128:(s + 1) * 128, :])
                            nc.vector.tensor_mul(xnt[:, s], xt_t, inv_b)

                    # -- phase 1: QKV + qk-norm + rope --
                    with (
                        tc.tile_pool(name="p1", bufs=2) as p1,
                        tc.tile_pool(name="p1c", bufs=1) as p1c,
                        tc.tile_pool(name="p1ps", bufs=2, space="PSUM") as p1ps,
                    ):
                        cq_sb = p1c.tile([128, c.T], BF16)
                        nc.sync.dma_start(cq_sb, cq[:])
                        sq_sb = p1c.tile([128, c.T], BF16)
                        nc.sync.dma_start(sq_sb, sq[:])
                        ck_sb = p1c.tile([128, c.T], BF16)
                        nc.sync.dma_start(ck_sb, ck[:])
                        sk_sb = p1c.tile([128, c.T], BF16)
                        nc.sync.dma_start(sk_sb, sk[:])
                        wv_sb = p1c.tile([128, c.HSUB, c.KVCOLS], BF16)
                        nc.sync.dma_start(
                            wv_sb, wvT[:].rearrange("(s p) q -> p s q", p=128))

                        def qk_head(dst, wT, h, cos_sb, sin_sb):
                            wh = p1.tile([128, c.HSUB, 128], BF16, tag="wh")
                            nc.sync.dma_start(
                                wh, wT[:].rearrange("(s p) q -> p s q", p=128)
                                [:, :, h * 128:(h + 1) * 128])
                            for q in range(c.T // c.TQ):
                                sl = slice(q * c.TQ, (q + 1) * c.TQ)
                                ps = p1ps.tile([128, c.TQ], F32, tag="qk_ps")
                                for s in range(c.HSUB):
                                    nc.tensor.matmul(
                                        ps, wh[:, s], xnt[:, s, sl],
                                        start=(s == 0),
                                        stop=(s == c.HSUB - 1))
                                qsq = p1.tile([128, c.TQ], F32, tag="qsq")
                                nc.scalar.activation(qsq, ps, AF.Square)
                                ssb = p1ps.tile([128, c.TQ], F32, tag="qk_ssq")
                                nc.tensor.matmul(ssb, ones_f, qsq,
                                                 start=True, stop=True)
                                rms = p1.tile([128, c.TQ], F32, tag="qk_rms")
                                nc.scalar.activation(rms, ssb, AF.Sqrt,
                                                     bias=eps_sb,
                                                     scale=1.0 / c.HD)
                                inv = p1.tile([128, c.TQ], F32, tag="qk_inv")
                                nc.vector.reciprocal(inv, rms)
                                qn = p1.tile([128, c.TQ], BF16, tag="qk_qn")
                                nc.vector.tensor_mul(qn, ps, inv)
                                qsw = p1.tile([128, c.TQ], BF16, tag="qk_qsw")
                                nc.sync.dma_start(qsw[0:64, :], qn[64:128, :])
                                nc.sync.dma_start(qsw[64:128, :], qn[0:64, :])
                                t1 = p1.tile([128, c.TQ], BF16, tag="qk_t1")
                                nc.vector.tensor_mul(t1, qn, cos_sb[:, sl])
                                t2 = p1.tile([128, c.TQ], BF16, tag="qk_t2")
                                nc.vector.tensor_mul(t2, qsw, sin_sb[:, sl])
                                nc.vector.tensor_add(dst[:, sl], t1, t2)

                        for h in range(c.QH):
                            qk_head(qbf[:, h], wqT, h, cq_sb, sq_sb)
                        for j in range(c.KVH):
                            qk_head(kbf[:, j], wkT, j, ck_sb, sk_sb)

                        for i in range(c.TT):
                            ps = p1ps.tile([128, c.KVCOLS], F32, tag="v_ps")
                            for s in range(c.HSUB):
                                nc.tensor.matmul(
                                    ps, xnt[:, s, i * 128:(i + 1) * 128],
                                    wv_sb[:, s],
                                    start=(s == 0), stop=(s == c.HSUB - 1))
                            nc.vector.tensor_copy(out=vbf[:, i], in_=ps)

                # -- phase 2: attention (xn pool closed) --
                with (
                    tc.tile_pool(name="p2", bufs=2) as p2,
                    tc.tile_pool(name="p2p", bufs=6) as p2p,
                    tc.tile_pool(name="p2ps", bufs=3, space="PSUM") as p2ps,
                    tc.tile_pool(name="p2psa", bufs=2, space="PSUM") as p2psa,
                ):
                    if c.mask_mode == "causal":
                        dm_sb = p2.tile([128, c.TPC, c.TQ], F32, bufs=1)
                        nc.sync.dma_start(
                            dm_sb, dmask[:].rearrange("(d p) q -> p d q", p=128))
                    for qc in range(c.NQC):
                        sl = slice(qc * c.TQ, (qc + 1) * c.TQ)
                        nk = n_tk(qc)
                        if c.mask_mode == "generic":
                            mk_sb = [p2.tile([128, c.TQ], F32, tag="mk",
                                             name=f"mk_{qc}_{i}", bufs=c.TT)
                                     for i in range(nk)]
                            for i in range(nk):
                                nc.sync.dma_start(
                                    mk_sb[i],
                                    maskT[i * 128:(i + 1) * 128, sl])
                        for h in range(c.QH):
                            jl = h // (c.QH // c.KVH)
                            den = p2psa.tile([128, c.TQ], F32, tag="den")
                            ctx = p2psa.tile([128, c.TQ], F32, tag="ctx")
                            for i in range(nk):
                                ps = p2ps.tile([128, c.TQ], F32, tag="s_ps")
                                nc.tensor.matmul(
                                    ps, kbf[:, jl, i * 128:(i + 1) * 128],
                                    qbf[:, h, sl], start=True, stop=True)
                                d = i - c.TPC * qc
                                if c.mask_mode == "causal" and d >= 0:
                                    nc.vector.tensor_add(ps, ps, dm_sb[:, d])
                                elif c.mask_mode == "generic":
                                    nc.vector.tensor_add(ps, ps, mk_sb[i])
                                pb = p2p.tile([128, c.TQ], BF16, tag="pbf")
                                nc.scalar.activation(pb, ps, AF.Exp)
                                nc.tensor.matmul(
                                    den, ones_b, pb,
                                    start=(i == 0), stop=(i == nk - 1))
                                nc.tensor.matmul(
                                    ctx, vbf[:, i, jl * 128:(jl + 1) * 128],
                                    pb, start=(i == 0), stop=(i == nk - 1))
                            invd = p2.tile([128, c.TQ], F32, tag="invd")
                            nc.vector.reciprocal(invd, den)
                            nc.vector.tensor_mul(ctxb[:, h, sl], ctx, invd)

                # -- phase 3: o-proj partial --
                with (
                    tc.tile_pool(name="p3", bufs=3) as p3,
                    tc.tile_pool(name="p3c", bufs=1) as p3c,
                    tc.tile_pool(name="p3ps", bufs=4, space="PSUM") as p3ps,
                ):
                    wo_sb = p3c.tile([128, c.QH, c.H], BF16)
                    nc.sync.dma_start(
                        wo_sb, woT[:].rearrange("(h p) m -> p h m", p=128))
                    for i in range(c.TT):
                        for m in range(c.H // 512):
                            ps = p3ps.tile([128, 512], F32, tag="o_ps")
                            for h in range(c.QH):
                                nc.tensor.matmul(
                                    ps, ctxb[:, h, i * 128:(i + 1) * 128],
                                    wo_sb[:, h, m * 512:(m + 1) * 512],
                                    start=(h == 0), stop=(h == c.QH - 1))
                            osb = p3.tile([128, 512], F32, tag="o_sb")
                            nc.vector.tensor_copy(out=osb, in_=ps)
                            nc.sync.dma_start(
                                o_part[i * 128:(i + 1) * 128,
                                       m * 512:(m + 1) * 512], osb)

            if no_cc:
                # timeline-sim variant: fake the ReduceScatter with a copy
                nc.sync.dma_start(o_red[:], o_part[0:c.TFFN, :])
            else:
                nc.gpsimd.collective_compute(
                    "ReduceScatter", ALU.add, replica_groups=groups,
                    ins=[o_part.opt()], outs=[o_red.opt()])

            # ---------------- phase 4: FFN (token-parallel) ----------------
            with (
                tc.tile_pool(name="p4x", bufs=1) as p4x,
                tc.tile_pool(name="p4", bufs=2) as p4,
                tc.tile_pool(name="p4h", bufs=1) as p4h,
            ):
                x2b = p4x.tile([128, c.TF, c.H], BF16)      # residual (bf16)
                x2nT = p4x.tile([128, c.HSUB, c.TFFN], BF16)
                nfch = c.DFF // 128
                fch_per = min(16, nfch)
                hT_parts = [
                    p4h.tile([128, fch_per, c.TFFN], BF16, tag="hT",
                             bufs=nfch // fch_per, name=f"hT_{i}")
                    for i in range(nfch // fch_per)
                ]

                def hT(fi):
                    return hT_parts[fi // fch_per][:, fi % fch_per]

                with tc.tile_pool(name="p4psa", bufs=2, space="PSUM") as p4psa:
                    for t in range(c.TF):
                        red = p4.tile([128, c.H], F32, tag="red", bufs=1)
                        nc.sync.dma_start(
                            red, o_red[t * 128:(t + 1) * 128, :])
                        res = p4.tile([128, c.H], F32, tag="res", bufs=1)
                        nc.sync.dma_start(
                            res, x_res[t * 128:(t + 1) * 128, :])
                        x2f = p4.tile([128, c.H], F32, tag="x2f", bufs=1)
                        nc.vector.tensor_add(x2f, red, res)
                        nc.vector.tensor_copy(out=x2b[:, t], in_=x2f)
                        sq_t = p4.tile([128, c.H], F32, tag="sq", bufs=1)
                        ssq = p4.tile([128, 1], F32, tag="ssq")
                        nc.scalar.activation(sq_t, x2f, AF.Square,
                                             accum_out=ssq)
                        rms = p4.tile([128, 1], F32, tag="rms1")
                        nc.scalar.activation(rms, ssq, AF.Sqrt,
                                             bias=eps_sb, scale=1.0 / c.H)
                        inv = p4.tile([128, 1], F32, tag="inv1")
                        nc.vector.reciprocal(inv, rms)
                        x2n = p4.tile([128, c.H], BF16, tag="x2n", bufs=1)
                        nc.vector.tensor_scalar_mul(x2n, x2f, inv)
                        for s in range(c.HSUB):
                            tp = p4psa.tile([128, 128], BF16, tag="tr_ps")
                            nc.tensor.transpose(
                                tp, x2n[:, s * 128:(s + 1) * 128], ident)
                            nc.vector.tensor_copy(
                                out=x2nT[:, s, t * 128:(t + 1) * 128], in_=tp)

                    # gate/up -> h^T
                    FO = 128
                    for fo in range(c.DFF // FO):
                        fsl = slice(fo * FO, (fo + 1) * FO)
                        wg_t = p4.tile([128, c.HSUB, FO], BF16, tag="wg")
                        nc.sync.dma_start(
                            wg_t,
                            wgT[:].rearrange("(s p) f -> p s f", p=128)
                            [:, :, fsl])
                        wu_t = p4.tile([128, c.HSUB, FO], BF16, tag="wu")
                        nc.sync.dma_start(
                            wu_t,
                            wuT[:].rearrange("(s p) f -> p s f", p=128)
                            [:, :, fsl])
                        for fl in range(FO // 128):
                            g_ps = p4psa.tile([128, c.TFFN], F32, tag="g_ps")
                            u_ps = p4psa.tile([128, c.TFFN], F32, tag="u_ps")
                            for s in range(c.HSUB):
                                nc.tensor.matmul(
                                    g_ps, wg_t[:, s, fl * 128:(fl + 1) * 128],
                                    x2nT[:, s],
                                    start=(s == 0), stop=(s == c.HSUB - 1))
                            for s in range(c.HSUB):
                                nc.tensor.matmul(
                                    u_ps, wu_t[:, s, fl * 128:(fl + 1) * 128],
                                    x2nT[:, s],
                                    start=(s == 0), stop=(s == c.HSUB - 1))
                            sig = p4.tile([128, c.TFFN], F32, tag="sig")
                            nc.scalar.activation(sig, g_ps, AF.Sigmoid)
                            su = p4.tile([128, c.TFFN], F32, tag="su")
                            nc.vector.tensor_mul(su, sig, u_ps)
                            nc.vector.tensor_mul(
                                hT(fo * (FO // 128) + fl), su, g_ps)

                # down + residual (separate PSUM pool scope)
                with tc.tile_pool(name="p4psd", bufs=c.TF + 2,
                                  space="PSUM") as p4psd:
                    for m in range(c.H // 512):
                        msl = slice(m * 512, (m + 1) * 512)
                        d_ps = [p4psd.tile([128, 512], F32, tag="d_ps",
                                           name=f"d_ps_{m}_{t}")
                                for t in range(c.TF)]
                        for fi in range(c.DFF // 128):
                            wd_t = p4.tile([128, 512], BF16, tag="wd", bufs=4)
                            nc.sync.dma_start(
                                wd_t, wdT[fi * 128:(fi + 1) * 128, msl])
                            for t in range(c.TF):
                                nc.tensor.matmul(
                                    d_ps[t],
                                    hT(fi)[:, t * 128:(t + 1) * 128], wd_t,
                                    start=(fi == 0),
                                    stop=(fi == c.DFF // 128 - 1))
                        for t in range(c.TF):
                            ob = p4.tile([128, 512], F32, tag="ob")
                            nc.vector.tensor_add(ob, d_ps[t], x2b[:, t, msl])
                            nc.sync.dma_start(
                                out[t * 128:(t + 1) * 128, msl], ob)

    nc.compile()
    return nc


def host_prep(cfg: Cfg, inputs: dict) -> list[dict]:
    """Build per-core input maps from the full problem inputs."""
    c = cfg
    f32 = np.float32
    x = np.asarray(inputs["x"], f32)
    anw = np.asarray(inputs["attn_norm_w"], f32)
    fnw = np.asarray(inputs["ffn_norm_w"], f32)
    qw = np.asarray(inputs["q_norm_w"], f32)
    kw = np.asarray(inputs["k_norm_w"], f32)
    w_q = np.asarray(inputs["w_q"], f32)
    w_k = np.asarray(inputs["w_k"], f32)
    w_v = np.asarray(inputs["w_v"], f32)
    w_o = np.asarray(inputs["w_o"], f32)
    w_gate = np.asarray(inputs["w_gate"], f32)
    w_up = np.asarray(inputs["w_up"], f32)
    w_down = np.asarray(inputs["w_down"], f32)
    rope_cos = np.asarray(inputs["rope_cos"], f32)
    rope_sin = np.asarray(inputs["rope_sin"], f32)

    scale = 1.0 / np.sqrt(float(c.HD))
    half = c.HD // 2
    cos, sin = rope_cos[:c.T], rope_sin[:c.T]           # [T, 64]
    ccatT = np.concatenate([cos, cos], axis=1).T         # [128, T]
    scatT = np.concatenate([-sin, sin], axis=1).T        # [128, T]
    qw_sw = np.roll(qw, -half)
    kw_sw = np.roll(kw, -half)
    cq = np.ascontiguousarray((ccatT * (qw * scale)[:, None]).astype(BF16NP))
    sq = np.ascontiguousarray((scatT * (qw_sw * scale)[:, None]).astype(BF16NP))
    ck = np.ascontiguousarray((ccatT * kw[:, None]).astype(BF16NP))
    sk = np.ascontiguousarray((scatT * kw_sw[:, None]).astype(BF16NP))

    wgTf = np.ascontiguousarray((w_gate * fnw[None, :]).T.astype(BF16NP))
    wuTf = np.ascontiguousarray((w_up * fnw[None, :]).T.astype(BF16NP))
    wdTf = np.ascontiguousarray(w_down.T.astype(BF16NP))

    dmask = maskT = None
    if c.mask_mode == "causal":
        p = np.arange(128)[:, None]
        f = np.arange(c.TQ)[None, :]
        dmask = np.concatenate(
            [np.where(p + 128 * d > f, NEG, 0.0).astype(f32)
             for d in range(c.TPC)], axis=0)
    elif c.mask_mode == "generic":
        am = np.asarray(inputs["attn_mask"], f32)
        maskT = np.ascontiguousarray(am.reshape(c.T, c.T).T, f32)

    in_maps = []
    for core in range(c.NCORES):
        b = core // c.GROUP
        g = core % c.GROUP
        xb = x[b]                                   # [T, H]
        qs = slice(g * c.QCOLS, (g + 1) * c.QCOLS)
        ks = slice(g * c.KVCOLS, (g + 1) * c.KVCOLS)
        ts = slice(g * c.TFFN, (g + 1) * c.TFFN)
        m = dict(
            xT=np.ascontiguousarray(xb.T, f32),
            x_res=np.ascontiguousarray(xb[ts], f32),
            wqT=np.ascontiguousarray((w_q[qs] * anw[None, :]).T.astype(BF16NP)),
            wkT=np.ascontiguousarray((w_k[ks] * anw[None, :]).T.astype(BF16NP)),
            wvT=np.ascontiguousarray((w_v[ks] * anw[None, :]).T.astype(BF16NP)),
            woT=np.ascontiguousarray(w_o[:, qs].T.astype(BF16NP)),
            wgT=wgTf, wuT=wuTf, wdT=wdTf,
            cq=cq, sq=sq, ck=ck, sk=sk,
        )
        if c.mask_mode == "causal":
            m["dmask"] = dmask
        elif c.mask_mode == "generic":
            m["maskT"] = maskT
        in_maps.append(m)
    return in_maps


def assemble(cfg: Cfg, results: list[dict]) -> np.ndarray:
    c = cfg
    B = c.NCORES // c.GROUP
    out = np.empty((B, c.T, c.H), np.float32)
    for core in range(c.NCORES):
        b = core // c.GROUP
        g = core % c.GROUP
        out[b, g * c.TFFN:(g + 1) * c.TFFN, :] = results[core]["out"]
    return out


def classify_mask(attn_mask: np.ndarray, T: int) -> str:
    m = np.asarray(attn_mask, np.float32).reshape(T, T)
    if not m.any():
        return "none"
    causal = np.triu(np.full((T, T), NEG, np.float32), k=1)
    if np.array_equal(m, causal):
        return "causal"
    return "generic"


_BUILD_CACHE: dict = {}


def _get_nc(cfg: Cfg):
    if cfg not in _BUILD_CACHE:
        _BUILD_CACHE[cfg] = build(cfg)
    return _BUILD_CACHE[cfg]


def kernel(**inputs) -> np.ndarray:
    from concourse.bass_utils import run_bass_kernel_spmd

    x = np.asarray(inputs["x"])
    B, T, H = x.shape
    DFF = inputs["w_gate"].shape[0]
    cfg = Cfg(T=T, H=H, DFF=DFF,
              mask_mode=classify_mask(inputs["attn_mask"], T))
    nc = _get_nc(cfg)
    in_maps = host_prep(cfg, inputs)
    res = run_bass_kernel_spmd(nc, in_maps, core_ids=list(range(cfg.NCORES)))
    return assemble(cfg, res.results)


if __name__ == "__main__":
    nc = build(Cfg())
    print("built + compiled OK")
